# revision 26
# baseline (speedup 1.0000x reference)
"""Trainium2 Bass kernel for nn_ChebyshevKANLayer (self-contained).

Math:
    xn   = 2*(x - rowmin)/(rowmax - rowmin) - 1          per row of x [8192,1024]
    T_j  = Chebyshev polynomials of xn, j=0..8
    y    = einsum('bij,ioj->bo', T, cheby_coeffs)        [8192, 1024]

Device algorithm (data-parallel over batch, 8 NeuronCores, 1024 rows each):
    - j=0 term folded into a host-computed bias[o] = sum_i C[i,o,0], added
      during the PSUM->SBUF epilogue.
    - P = 2*xn is computed in fp16 and PE-transposed to put the contraction
      index i on SBUF partitions.  C_1 is halved on the host so P itself is
      the j=1 matmul operand.
    - T_2..T_8 computed by the Chebyshev recurrence on the vector engine in
      fp16 (T_n = P*T_{n-1} - T_{n-2}; T_3 fused to one op), pipelined one
      degree ahead of the matmul stream.
    - y[b,o] accumulated in PSUM (fp32): stationary = T_j chunk [128i,128b],
      moving = coeffs [128i,512o]; 1024 fp16 matmuls of N=512 per core,
      accumulating over all 64 (j,i)-chunks per output tile.
"""

import numpy as np

B, I, O, DEG = 8192, 1024, 1024, 8
NCORES = 8
BC = B // NCORES          # 1024 batch rows per core
BT = 128                  # batch tile (partitions) for stage A
NBT = BC // BT            # 8
BCW = 256                 # T-plane chunk width (batch)
NBCH = BC // BCW          # 4
NBS = BCW // 128          # 2 batch sub-chunks (stationary M) per chunk
OH = 512                  # matmul moving width over output dim
NOH = O // OH             # 2
NIB = I // 128            # 8 input-dim chunks of 128

_CACHE = {}


def _build_program():
    import concourse.bacc as bacc
    import concourse.mybir as mybir
    import concourse.tile as tile
    from contextlib import ExitStack

    f32 = mybir.dt.float32
    f16 = mybir.dt.float16
    Alu = mybir.AluOpType
    AX = mybir.AxisListType

    nc = bacc.Bacc("TRN2", target_bir_lowering=False, debug=False, num_devices=1)

    x_d = nc.dram_tensor("x_shard", [BC, I], f16, kind="ExternalInput")
    c_d = nc.dram_tensor("coeffs_t", [DEG, I, O], f16, kind="ExternalInput")
    b_d = nc.dram_tensor("bias_bc", [128, O], f16, kind="ExternalInput")
    i_d = nc.dram_tensor("ident", [128, 128], f16, kind="ExternalInput")
    y_d = nc.dram_tensor("y_out", [BC, O], f32, kind="ExternalOutput")

    with tile.TileContext(nc) as tc, ExitStack() as ctx:
        const_pool = ctx.enter_context(tc.tile_pool(name="const", bufs=1))
        cpool = ctx.enter_context(tc.tile_pool(name="cpool", bufs=1))
        ppool = ctx.enter_context(tc.tile_pool(name="ppool", bufs=1))
        xpool = ctx.enter_context(tc.tile_pool(name="xpool", bufs=2))
        spool = ctx.enter_context(tc.tile_pool(name="spool", bufs=2))
        tpool = ctx.enter_context(tc.tile_pool(name="tpool", bufs=1))
        gpool = ctx.enter_context(tc.tile_pool(name="gpool", bufs=2))
        pacc = ctx.enter_context(tc.tile_pool(name="pacc", bufs=1, space="PSUM"))
        ptr = ctx.enter_context(tc.tile_pool(name="ptr", bufs=3, space="PSUM"))

        id_sb = const_pool.tile([128, 128], f16)
        nc.sync.dma_start(id_sb[:], i_d.ap())
        bias_sb = const_pool.tile([128, O], f16)
        nc.sync.dma_start(bias_sb[:], b_d.ap())

        # DMA emission order: x tiles for the first chunks ahead of the
        # 16.8 MB coefficient stream, remaining x tiles interleaved between
        # early coefficient planes.
        # All input DMAs go through the sync engine's single queue: the queue
        # is strict FIFO (sharded over all 16 DMA engines at full HBM
        # bandwidth), so emission order here is an exact priority order —
        # round-robin across multiple queues would dilute the critical first
        # transfers.  Output DMAs issue from the (otherwise idle) scalar
        # sequencer.
        x_tiles = [None] * NBT

        def load_x(bt):
            x_t = xpool.tile([128, I], f16, tag=f"x{bt}", name=f"x_{bt}", bufs=1)
            nc.sync.dma_start(x_t[:], x_d.ap()[bt * BT:(bt + 1) * BT, :])
            x_tiles[bt] = x_t

        # Coefficients resident in SBUF, split per j-plane into a few tiles:
        # fine enough that early matmuls only wait for their own chunk,
        # coarse enough to keep DMA instruction count low.  The j=1 plane
        # (needed first) is split finer.
        C_t = [None] * DEG

        def load_c(j):
            nib_per = 1 if j == 0 else 4
            tiles = []
            for h in range(NIB // nib_per):
                ct = cpool.tile(
                    [128, nib_per, O], f16, tag=f"C{j}_{h}", name=f"C_{j}_{h}"
                )
                lo_i = h * nib_per * 128
                nc.sync.dma_start(
                    ct[:],
                    c_d.ap()[j, lo_i:lo_i + nib_per * 128, :].rearrange(
                        "(ib p) o -> p ib o", p=128
                    ),
                )
                tiles.append(ct)
            C_t[j] = (tiles, nib_per)

        # Priority order: first chunk's x tiles, then coefficients for the
        # first matmul layers, with later x tiles slotted in well before
        # their chunk starts.
        load_x(0)
        load_x(1)
        load_c(0)
        load_c(1)
        load_x(2)
        load_x(3)
        load_c(2)
        load_x(4)
        load_x(5)
        load_c(3)
        load_x(6)
        load_x(7)
        for j in range(4, DEG):
            load_c(j)

        # P = 2*xn, transposed: [i_in, i_blk, b] fp16
        P_buf = ppool.tile([128, NIB, BC], f16)

        # PE warm-up: the HAM clock gate holds the PE at 1.2 GHz until it has
        # been busy ~3.4us.  The PE is idle during the DMA/normalize prologue
        # (~7..17us), so run dummy matmuls on a zeroed tile into a scratch
        # PSUM bank, sized to end just before the real stream starts — the
        # first real matmuls then issue at the full 2.4 GHz.
        dummy_sb = const_pool.tile([128, 512], f16)
        nc.gpsimd.memset(dummy_sb[:], 0.0)
        dummy_ps = ptr.tile([128, OH], f32, tag="warm", bufs=1)
        for w in range(40):
            nc.tensor.matmul(
                dummy_ps[:], dummy_sb[:, :128], dummy_sb[:, :OH],
                start=(w == 0), stop=(w == 39),
            )

        def stage_a(bt):
            """Normalize x tile to P=2*xn (fp16), transpose into P_buf."""
            x_t = x_tiles[bt]
            mx = spool.tile([128, 1], f32, tag="mx", name=f"mx_{bt}")
            mn = spool.tile([128, 1], f32, tag="mn", name=f"mn_{bt}")
            if bt < 2:
                # Critical path (first chunk): tree-reduce in fp16 (cheap
                # tensor_tensor stages) instead of two 1.2us full-width
                # reduces, and keep ops short so the dependent scalar chain
                # isn't head-of-line blocked on the vector engine.
                h1 = spool.tile([128, 512], f16, tag="h1", name=f"h1_{bt}", bufs=1)
                nc.vector.tensor_tensor(
                    h1[:], x_t[:, :512], x_t[:, 512:], op=Alu.max
                )
                h2 = spool.tile([128, 256], f16, tag="h2", name=f"h2_{bt}", bufs=1)
                nc.vector.tensor_tensor(
                    h2[:], h1[:, :256], h1[:, 256:], op=Alu.max
                )
                nc.vector.tensor_reduce(mx[:], h2[:], axis=AX.X, op=Alu.max)
                g1 = spool.tile([128, 512], f16, tag="g1", name=f"g1_{bt}", bufs=1)
                nc.vector.tensor_tensor(
                    g1[:], x_t[:, :512], x_t[:, 512:], op=Alu.min
                )
                g2 = spool.tile([128, 256], f16, tag="g2", name=f"g2_{bt}", bufs=1)
                nc.vector.tensor_tensor(
                    g2[:], g1[:, :256], g1[:, 256:], op=Alu.min
                )
                nc.vector.tensor_reduce(mn[:], g2[:], axis=AX.X, op=Alu.min)
            else:
                nc.vector.tensor_reduce(mx[:], x_t[:], axis=AX.X, op=Alu.max)
                nc.vector.tensor_reduce(mn[:], x_t[:], axis=AX.X, op=Alu.min)
            rng = spool.tile([128, 1], f32, tag="rng", name=f"rng_{bt}")
            nc.vector.tensor_sub(rng[:], mx[:], mn[:])
            rcp = spool.tile([128, 1], f32, tag="rcp", name=f"rcp_{bt}")
            nc.vector.reciprocal(rcp[:], rng[:])
            s2 = spool.tile([128, 1], f32, tag="s2", name=f"s2_{bt}")
            nc.vector.tensor_scalar_mul(s2[:], rcp[:], 4.0)
            # t2 = -4*min*rcp - 2
            t2a = spool.tile([128, 1], f32, tag="t2a", name=f"t2a_{bt}")
            nc.vector.scalar_tensor_tensor(
                t2a[:], mn[:], -4.0, rcp[:], op0=Alu.mult, op1=Alu.mult
            )
            t2b = spool.tile([128, 1], f32, tag="t2b", name=f"t2b_{bt}")
            nc.vector.tensor_scalar_add(t2b[:], t2a[:], -2.0)
            # P_nat = x*s2 + t2   (= 2*xn), fp16
            pn = xpool.tile([128, I], f16, tag="pn", name=f"pn_{bt}")
            nc.vector.tensor_scalar(
                pn[:], x_t[:], s2[:], t2b[:], op0=Alu.mult, op1=Alu.add
            )
            for ib in range(NIB):
                ps = ptr.tile([128, 128], f16, tag="ps", name=f"ps_{bt}_{ib}")
                nc.tensor.transpose(ps[:], pn[:, ib * 128:(ib + 1) * 128], id_sb[:])
                nc.scalar.copy(P_buf[:, ib, bt * BT:(bt + 1) * BT], ps[:])

        def emit_T(bc, n, Tp):
            """Emit DVE ops producing T_n plane [128, NIB, BCW] for chunk bc."""
            lo = bc * BCW
            Tn = tpool.tile([128, NIB, BCW], f16, tag=f"T{n}", name=f"T{n}_{bc}")
            if n >= 4 or n == 2:
                tmp = tpool.tile(
                    [128, NIB, BCW], f16, tag="tmp", name=f"tmp{n}_{bc}", bufs=1
                )
            for ib in range(NIB):
                Ps = P_buf[:, ib, lo:lo + BCW]
                if n == 2:
                    # T2 = 0.5*P*P - 1
                    nc.vector.scalar_tensor_tensor(
                        tmp[:, ib, :], Ps, 0.5, Ps, op0=Alu.mult, op1=Alu.mult
                    )
                    nc.vector.tensor_scalar_add(Tn[:, ib, :], tmp[:, ib, :], -1.0)
                elif n == 3:
                    # T3 = (T2 - 0.5) * P
                    nc.vector.scalar_tensor_tensor(
                        Tn[:, ib, :], Tp[2][:, ib, :], -0.5, Ps,
                        op0=Alu.add, op1=Alu.mult,
                    )
                else:
                    nc.vector.tensor_mul(tmp[:, ib, :], Ps, Tp[n - 1][:, ib, :])
                    nc.vector.tensor_sub(
                        Tn[:, ib, :], tmp[:, ib, :], Tp[n - 2][:, ib, :]
                    )
            Tp[n] = Tn

        for bc in range(NBCH):
            stage_a(2 * bc)
            stage_a(2 * bc + 1)

            lo = bc * BCW
            # 4 accumulators (one PSUM bank each): index 2*bs + oh
            accs = [
                pacc.tile([128, OH], f32, tag=f"acc{p}", name=f"acc{p}_{bc}")
                for p in range(NBS * NOH)
            ]
            Tp = {}

            def mk_sta(j, ib, bs):
                # stationary: [128 i, 128 b] slice of T_j (P for j=1)
                if j == 1:
                    return P_buf[:, ib, lo + bs * 128:lo + (bs + 1) * 128]
                return Tp[j][:, ib, bs * 128:(bs + 1) * 128]

            def mk_mov(j, ib, oh):
                tiles, nib_per = C_t[j - 1]
                return tiles[ib // nib_per][
                    :, ib % nib_per, oh * OH:(oh + 1) * OH
                ]

            for j in range(1, DEG + 1):
                if j + 1 <= DEG:
                    emit_T(bc, j + 1, Tp)
                if j < DEG:
                    for ib in range(NIB):
                        for bs in range(NBS):
                            sta = mk_sta(j, ib, bs)
                            for oh in range(NOH):
                                nc.tensor.matmul(
                                    accs[NOH * bs + oh][:],
                                    sta,
                                    mk_mov(j, ib, oh),
                                    start=(j == 1 and ib == 0),
                                    stop=False,
                                )
                else:
                    # Last layer: finish one PSUM bank at a time so its
                    # epilogue overlaps the other banks' matmuls.
                    for bs in range(NBS):
                        for oh in range(NOH):
                            for ib in range(NIB):
                                nc.tensor.matmul(
                                    accs[NOH * bs + oh][:],
                                    mk_sta(j, ib, bs),
                                    mk_mov(j, ib, oh),
                                    start=False,
                                    stop=(ib == NIB - 1),
                                )
            # Epilogue: bias add (fp32) + store y chunk.
            for bs in range(NBS):
                for oh in range(NOH):
                    stg = gpool.tile(
                        [128, OH], f32, tag="stg", name=f"stg_{bc}_{bs}_{oh}"
                    )
                    nc.vector.tensor_add(
                        stg[:],
                        accs[NOH * bs + oh][:],
                        bias_sb[:, oh * OH:(oh + 1) * OH],
                    )
                    nc.scalar.dma_start(
                        y_d.ap()[
                            lo + bs * 128:lo + (bs + 1) * 128,
                            oh * OH:(oh + 1) * OH,
                        ],
                        stg[:],
                    )

    nc.compile()
    return nc


def _prep_inputs(x, cheby_coeffs):
    x = np.ascontiguousarray(np.asarray(x, dtype=np.float32))
    C = np.asarray(cheby_coeffs, dtype=np.float32)
    assert x.shape == (B, I) and C.shape == (I, O, DEG + 1)

    bias = C[:, :, 0].sum(axis=0, dtype=np.float64).astype(np.float32)  # [O]
    bias_bc = np.ascontiguousarray(
        np.broadcast_to(bias[None, :], (128, O)).astype(np.float16)
    )

    Ct = np.moveaxis(C[:, :, 1:], 2, 0).copy()                          # [DEG, I, O]
    Ct[0] *= 0.5                                                        # P = 2*xn carries j=1
    Ct16 = np.ascontiguousarray(Ct.astype(np.float16))

    ident = np.eye(128, dtype=np.float16)
    shards = x.reshape(NCORES, BC, I).astype(np.float16)
    in_maps = [
        {
            "x_shard": np.ascontiguousarray(shards[c]),
            "coeffs_t": Ct16,
            "bias_bc": bias_bc,
            "ident": ident,
        }
        for c in range(NCORES)
    ]
    return in_maps


def _run(in_maps, trace=False):
    from concourse import bass_utils

    if "nc" not in _CACHE:
        _CACHE["nc"] = _build_program()
    nc = _CACHE["nc"]
    res = None
    for attempt in range(3):
        try:
            res = bass_utils.run_bass_kernel_spmd(
                nc, in_maps, list(range(NCORES)), trace=trace
            )
            break
        except Exception:
            # Rare transient NRT device errors recover on retry.
            if attempt == 2:
                raise
    y = np.empty((B, O), dtype=np.float32)
    for c in range(NCORES):
        y[c * BC:(c + 1) * BC, :] = res.results[c]["y_out"]
    return y, res


def kernel(x, cheby_coeffs):
    in_maps = _prep_inputs(x, cheby_coeffs)
    y, _ = _run(in_maps, trace=False)
    return y
